# revision 30
# baseline (speedup 1.0000x reference)
"""Block2D attention on 8 TRN2 NeuronCores (fp16 compute, fp32 accum).

Sharding: data-parallel over the 8 independent (b, bnx, bny) attention blocks
(B=2 x bnx=2 x bny=2), one block of T=1024 tokens per core. Blocks are fully
independent so no collectives are needed; each core runs the whole
qkv-projection -> block attention -> output projection chain for its block.

Exploits PE array row-tiling concurrency for QK^T: the two heads of a pair
use K=64 contractions on disjoint SBUF partition halves (row groups (0,0)
and (64,0)); emitted back-to-back into disjoint psum banks of one
[128, 2, 512] tile they co-stream on the PE (dispatch ~4ns apart), halving
QK wall time. A single ACT exp drains both heads' scores per (t, n).

The pair loop is software-pipelined: PV runs as two sequential per-head
sweeps (head 1 of pair j-1 during slots 0-3, head 0 of pair j during slots
4-7) so PV holds only 2 psum banks, freeing a dedicated Qproj psum tag —
Qproj(j+1) then executes mid-pair as PE fill while ACT drains exps (a shared
psum ring instead serializes Qproj at pair boundaries and starves ACT
~6.5us/pair). The last pair's head-1 sweep runs inside its own t-loop on the
idle Qproj psum so no serial epilogue delays the output projection. Setup
runs V-proj k-outer with 8 live psum accumulators so matmuls start as soon
as the first xT/Wv k-tile DMA lands.

PSUM budget (8 banks): qk 2 x [128,2,512] (4) + pv 2 x [65,512] (2) +
pp/Qproj 2 x [128,512] (2).
"""

import os
import sys

sys.path.insert(0, "/opt/trn_rl_repo")

import numpy as np
import ml_dtypes

import concourse.bass as bass
from concourse import bacc
import concourse.mybir as mybir
import concourse.tile as tile

F32 = mybir.dt.float32
BF16 = mybir.dt.float16   # compute dtype: fp16 (same PE speed as bf16, 8x finer mantissa)
BF = np.float16

H = 2048        # hidden
T = 1024        # tokens per block
NH = 32         # q heads
NKV = 8         # kv heads
D = 64          # head dim
KT = H // 128   # 16 hidden k-tiles
TT = T // 128   # 8 token tiles
NPAIR = NH // 2  # 16 head pairs
SCALE = D ** -0.5

LAST_EXEC_TIME_NS = None
LAST_RESULTS = None
_CACHED_NC = None


def build_nc(e_bufs=20, wq_bufs=2):
    nc = bacc.Bacc("TRN2")
    xT = nc.dram_tensor("xT", [H, T], BF16, kind="ExternalInput")
    wq = nc.dram_tensor("wq", [H, H], BF16, kind="ExternalInput")
    wk = nc.dram_tensor("wk", [H, NKV * D], BF16, kind="ExternalInput")
    wv = nc.dram_tensor("wv", [H, NKV * D], BF16, kind="ExternalInput")
    wo = nc.dram_tensor("wo", [H, H], BF16, kind="ExternalInput")
    out = nc.dram_tensor("out", [T, H], F32, kind="ExternalOutput")
    scr = nc.dram_tensor("scr", [NPAIR, 2, 2, 512], F32)  # recip bounce [j, hh, n]

    xT_v = xT.ap().rearrange("(k p) t -> p k t", p=128)
    wq_v = wq.ap().rearrange("(k p) m -> p k m", p=128)
    wk_v = wk.ap().rearrange("(k p) m -> p k m", p=128)
    wv_v = wv.ap().rearrange("(k p) m -> p k m", p=128)
    wo_v = wo.ap().rearrange("(k p) m -> p k m", p=128)

    with tile.TileContext(nc) as tc:
        with (
            tc.tile_pool(name="oT", bufs=1) as oT_pool,
            tc.tile_pool(name="xTs", bufs=1) as xT_pool,
            tc.tile_pool(name="kdup", bufs=1) as kdup_pool,
            tc.tile_pool(name="vplus", bufs=1) as vplus_pool,
            tc.tile_pool(name="wos", bufs=2) as wo_pool,
        ):
            oT = oT_pool.tile([128, KT, T], BF16)
            xTs = xT_pool.tile([128, KT, T], BF16)
            kdup = kdup_pool.tile([128, NKV, T], BF16)  # k_g^T on both halves
            vplus = vplus_pool.tile([128, TT, NKV, D + 1], BF16)

            # ---------------- setup: v and k projections ----------------
            # k-outer V projection with 8 live psum accumulators: matmuls
            # start as soon as the first (xT, wv) k-tile DMA lands.
            with (
                tc.tile_pool(name="wvs", bufs=1) as wv_pool,
                tc.tile_pool(name="wks", bufs=2) as wk_pool,
                tc.tile_pool(name="kTs", bufs=1) as kT_pool,
                tc.tile_pool(name="sps", bufs=8, space="PSUM") as sps,
            ):
                wv_s = wv_pool.tile([128, KT, 512], BF16)
                wk_s0 = wk_pool.tile([128, KT, 128], BF16, tag="wk_s")
                for k in range(KT):
                    nc.sync.dma_start(out=xTs[:, k:k + 1, :], in_=xT_v[:, k:k + 1, :])
                    nc.sync.dma_start(out=wv_s[:, k:k + 1, :], in_=wv_v[:, k:k + 1, :])
                    nc.sync.dma_start(
                        out=wk_s0[:, k:k + 1, :], in_=wk_v[:, k:k + 1, 0:128]
                    )
                # merged k-loop: 6 V-proj accumulators + K-proj chunk 0's two
                # chains share the pass, so kdup heads g0/g1 exist (and pair 0
                # can start) without waiting for a separate K-proj phase.
                vps = [
                    sps.tile([128, 512], F32, tag="sps", name=f"vps{m}")
                    for m in range(6)
                ]
                kps = [
                    sps.tile([128, 512], F32, tag="sps", name=f"kps{n}")
                    for n in range(2)
                ]
                for k in range(KT):
                    for m in range(6):
                        nc.tensor.matmul(
                            vps[m], xTs[:, k, 128 * m:128 * (m + 1)], wv_s[:, k, :],
                            start=(k == 0), stop=(k == KT - 1),
                        )
                    for n in range(2):
                        nc.tensor.matmul(
                            kps[n], wk_s0[:, k, :], xTs[:, k, 512 * n:512 * (n + 1)],
                            start=(k == 0), stop=(k == KT - 1),
                        )
                kTs = kT_pool.tile([128, 4, T], BF16)
                for n in range(2):
                    nc.vector.tensor_copy(kTs[:, 0, 512 * n:512 * (n + 1)], kps[n])
                for g in (0, 1):
                    src = kTs[64 * (g % 2):64 * (g % 2) + 64, 0, :]
                    nc.sync.dma_start(out=kdup[0:64, g, :], in_=src)
                    nc.sync.dma_start(out=kdup[64:128, g, :], in_=src)
                for m in range(6):
                    nc.vector.tensor_copy(
                        vplus[:, m, :, 0:D],
                        vps[m].rearrange("p (h d) -> p h d", h=NKV),
                    )
                for m in (6, 7):
                    ps = sps.tile([128, 512], F32, tag="sps", name=f"vps{m}")
                    for k in range(KT):
                        nc.tensor.matmul(
                            ps, xTs[:, k, 128 * m:128 * (m + 1)], wv_s[:, k, :],
                            start=(k == 0), stop=(k == KT - 1),
                        )
                    nc.vector.tensor_copy(
                        vplus[:, m, :, 0:D],
                        ps.rearrange("p (h d) -> p h d", h=NKV),
                    )
                nc.vector.memset(vplus[:, :, :, D:D + 1], 1.0)

                for m in range(1, 4):
                    wk_s = wk_pool.tile([128, KT, 128], BF16, tag="wk_s")
                    nc.sync.dma_start(out=wk_s, in_=wk_v[:, :, 128 * m:128 * (m + 1)])
                    for n in range(2):
                        ps = sps.tile([128, 512], F32, tag="sps")
                        for k in range(KT):
                            nc.tensor.matmul(
                                ps, wk_s[:, k, :], xTs[:, k, 512 * n:512 * (n + 1)],
                                start=(k == 0), stop=(k == KT - 1),
                            )
                        nc.vector.tensor_copy(kTs[:, m, 512 * n:512 * (n + 1)], ps)
                    # duplicate the two kv heads of this chunk onto both
                    # partition halves as soon as they exist, so early pairs'
                    # QK can start while later K-proj chunks still run.
                    for g in (2 * m, 2 * m + 1):
                        src = kTs[64 * (g % 2):64 * (g % 2) + 64, g // 2, :]
                        nc.sync.dma_start(out=kdup[0:64, g, :], in_=src)
                        nc.sync.dma_start(out=kdup[64:128, g, :], in_=src)

            # ---------------- pair loop ----------------
            # Per t-slot: QK co-pairs (row-tiled, co-streaming) + exp, plus
            # PV fill work. PV runs as two sequential per-head sweeps so only
            # 2 psum banks are held by PV at any time: head 1 of pair j-1
            # during slots 0-3, head 0 of pair j during slots 4-7. This frees
            # 2 banks for a dedicated Qproj psum tag so Qproj(j+1) runs truly
            # concurrent mid-pair (a shared ring serializes at boundaries and
            # starves ACT ~6.5us/pair).
            with (
                tc.tile_pool(name="wqs", bufs=wq_bufs) as wq_pool,
                tc.tile_pool(name="qpair", bufs=2) as qpair_pool,
                tc.tile_pool(name="e", bufs=e_bufs) as e_pool,
                tc.tile_pool(name="ou", bufs=4) as ou_pool,
                tc.tile_pool(name="rec", bufs=2) as rec_pool,
                tc.tile_pool(name="rbc", bufs=2) as rbc_pool,
                tc.tile_pool(name="qk", bufs=2, space="PSUM") as qk_pool,
                tc.tile_pool(name="pv", bufs=2, space="PSUM") as pv_pool,
                tc.tile_pool(name="pp", bufs=2, space="PSUM") as pp_pool,
            ):
                wo_tiles = [None] * 4
                wo_s0 = wo_pool.tile([128, KT, 512], BF16, tag="wo_s")
                nc.sync.dma_start(out=wo_s0, in_=wo_v[:, :, 0:512])
                wo_tiles[0] = wo_s0

                def emit_qproj(j):
                    """Q projection for pair j -> qpair tile [128, T]."""
                    wq_s = wq_pool.tile([128, KT, 128], BF16, tag="wq_s")
                    nc.sync.dma_start(out=wq_s, in_=wq_v[:, :, 128 * j:128 * (j + 1)])
                    qpair = qpair_pool.tile([128, T], BF16, tag="qpair")
                    for n in range(2):
                        pp = pp_pool.tile([128, 512], F32, tag="pp")
                        for k in range(KT):
                            nc.tensor.matmul(
                                pp, wq_s[:, k, :], xTs[:, k, 512 * n:512 * (n + 1)],
                                start=(k == 0), stop=(k == KT - 1),
                            )
                        nc.vector.tensor_copy(qpair[:, 512 * n:512 * (n + 1)], pp)
                    return qpair

                def emit_evac(j, hh, pv_pair):
                    """Evacuate one head's PV chains: unnormalized o to the
                    pair-packed oT layout, normalized by 1/den (recip +
                    DRAM-bounce broadcast + gpsimd mul, off critical path)."""
                    lo, hi = 64 * hh, 64 * hh + 64
                    # o data sits on partitions 64hh..64hh+64 so the gpsimd
                    # muls see equal base partitions on all three operands.
                    ou_t = ou_pool.tile([128, 2, 512], F32, tag="ou",
                                        name=f"ou_{j}_{hh}")
                    den = rec_pool.tile([1, 2 * 512], F32, tag="den")
                    for n in range(2):
                        nc.vector.tensor_copy(
                            ou_t[lo:hi, n, :], pv_pair[n][0:D, :]
                        )
                        nc.vector.tensor_copy(
                            den[:, 512 * n:512 * (n + 1)], pv_pair[n][D:D + 1, :]
                        )
                    # spread den over 128 partitions so the reciprocal runs
                    # all lanes wide.
                    den_sp = rec_pool.tile([128, 8], F32, tag="den_sp")
                    sp_src = bass.AP(
                        tensor=den.tensor, offset=den.offset,
                        ap=[[1, 1], [8, 128], [1, 8]],
                    )
                    nc.sync.dma_start(out=den_sp, in_=sp_src)
                    rec_sp = rec_pool.tile([128, 8], F32, tag="rec_sp")
                    nc.vector.reciprocal(rec_sp, den_sp)
                    nc.sync.dma_start(
                        out=scr.ap()[j, hh].rearrange("a b -> (a b)")
                        .rearrange("(p e) -> p e", p=128),
                        in_=rec_sp,
                    )
                    rbc = rbc_pool.tile([128, T], F32, tag="rbc")
                    bsrc = bass.AP(
                        tensor=scr.ap().tensor,
                        offset=(2 * j + hh) * T,
                        ap=[[0, 64], [1, 1024]],
                    )
                    nc.sync.dma_start(out=rbc[lo:hi, :], in_=bsrc)
                    for n in range(2):
                        nc.gpsimd.tensor_mul(
                            oT[lo:hi, j, 512 * n:512 * (n + 1)],
                            ou_t[lo:hi, n, :],
                            rbc[lo:hi, 512 * n:512 * (n + 1)],
                        )

                def pv_sweep_steps(j, hh, pv_pair, e_tiles, s0, s1,
                                   pool=None, ptag="pv"):
                    g = j // 2
                    for s in range(s0, s1):
                        for n in range(2):
                            if pv_pair[n] is None:
                                pv_pair[n] = (pool or pv_pool).tile(
                                    [D + 1, 512], F32, tag=ptag,
                                    name=f"pv_{j}_{hh}_{n}",
                                )
                            nc.tensor.matmul(
                                pv_pair[n], vplus[:, s, g, :],
                                e_tiles[s][n][:, hh, :],
                                start=(s == 0), stop=(s == TT - 1),
                            )

                qpair_next = emit_qproj(0)
                e_prev = None      # pair j-1's e tiles, consumed by its h1 sweep
                pv_h1 = [None, None]
                for j in range(NPAIR):
                    g = j // 2
                    qpair = qpair_next
                    e_tiles = [[None] * 2 for _ in range(TT)]  # [t][n]
                    pv_h0 = [None, None]

                    for t in range(TT):
                        for n in range(2):
                            qk = qk_pool.tile([128, 2, 512], F32, tag="qk")
                            for hh in range(2):
                                lo, hi = 64 * hh, 64 * hh + 64
                                nc.tensor.matmul(
                                    qk[:, hh, :],
                                    kdup[lo:hi, g, 128 * t:128 * (t + 1)],
                                    qpair[lo:hi, 512 * n:512 * (n + 1)],
                                    start=True, stop=True,
                                )
                            e = e_pool.tile([128, 2, 512], BF16, tag="e")
                            nc.scalar.activation(
                                e, qk, mybir.ActivationFunctionType.Exp, scale=SCALE
                            )
                            e_tiles[t][n] = e
                        if t < 4:
                            # head-1 sweep of the previous pair (2 steps/slot)
                            if e_prev is not None:
                                pv_sweep_steps(j - 1, 1, pv_h1, e_prev,
                                               2 * t, 2 * t + 2)
                                if t == 3:
                                    emit_evac(j - 1, 1, pv_h1)
                                    pv_h1 = [None, None]
                        else:
                            # head-0 sweep of this pair (2 steps/slot)
                            pv_sweep_steps(j, 0, pv_h0, e_tiles,
                                           2 * (t - 4), 2 * (t - 4) + 2)
                            if t == 4 and j + 1 < NPAIR:
                                qpair_next = emit_qproj(j + 1)
                            if j == NPAIR - 1:
                                # last pair: its head-1 sweep runs inside the
                                # t-loop on the idle Qproj psum (no Qproj(16)),
                                # so no serial epilogue delays the O-proj.
                                pv_sweep_steps(j, 1, pv_h1, e_tiles,
                                               2 * (t - 4), 2 * (t - 4) + 2,
                                               pool=pp_pool, ptag="pp")
                    emit_evac(j, 0, pv_h0)
                    if j == NPAIR - 1:
                        emit_evac(j, 1, pv_h1)
                    e_prev = e_tiles
                    pv_h1 = [None, None]

            # ---------------- output projection ----------------
            with (
                tc.tile_pool(name="ob", bufs=4) as ob_pool,
                tc.tile_pool(name="ops", bufs=4, space="PSUM") as ops,
            ):
                for c in range(4):
                    if wo_tiles[c] is None:
                        wo_s = wo_pool.tile([128, KT, 512], BF16, tag="wo_s")
                        nc.sync.dma_start(
                            out=wo_s, in_=wo_v[:, :, 512 * c:512 * (c + 1)]
                        )
                        wo_tiles[c] = wo_s
                    wo_s = wo_tiles[c]
                    for m in range(TT):
                        ps = ops.tile([128, 512], F32, tag="ops")
                        for k in range(KT):
                            nc.tensor.matmul(
                                ps, oT[:, k, 128 * m:128 * (m + 1)], wo_s[:, k, :],
                                start=(k == 0), stop=(k == KT - 1),
                            )
                        ob = ob_pool.tile([128, 512], F32, tag="ob")
                        nc.scalar.copy(ob, ps)
                        nc.sync.dma_start(
                            out=out.ap()[128 * m:128 * (m + 1), 512 * c:512 * (c + 1)],
                            in_=ob,
                        )
    nc.finalize()
    return nc


def _prep_inputs(hidden_states, Wq, Wk, Wv, Wo):
    hs = np.asarray(hidden_states, dtype=np.float32)
    B = hs.shape[0]
    # token index l = ix*2048 + sx*64 + iy*32 + sy  (bnx=2, BSX=32, bny=2, BSY=32)
    hsv = hs.reshape(B, 2, 32, 2, 32, H)  # b ix sx iy sy h
    wq_b = np.asarray(Wq, dtype=np.float32).astype(BF)
    wk_b = np.asarray(Wk, dtype=np.float32).astype(BF)
    wv_b = np.asarray(Wv, dtype=np.float32).astype(BF)
    wo_b = np.asarray(Wo, dtype=np.float32).astype(BF)
    in_maps = []
    for c in range(8):
        b, ix, iy = c // 4, (c // 2) % 2, c % 2
        x_blk = hsv[b, ix, :, iy, :, :].reshape(T, H)
        xT = np.ascontiguousarray(x_blk.T).astype(BF)
        in_maps.append({"xT": xT, "wq": wq_b, "wk": wk_b, "wv": wv_b, "wo": wo_b})
    return in_maps


def kernel(hidden_states, Wq, Wk, Wv, Wo, x_dim=64, y_dim=64):
    global LAST_EXEC_TIME_NS, LAST_RESULTS, _CACHED_NC
    assert int(x_dim) == 64 and int(y_dim) == 64

    from concourse.bass_utils import run_bass_kernel_spmd

    if _CACHED_NC is None:
        _CACHED_NC = build_nc()
    nc = _CACHED_NC

    in_maps = _prep_inputs(hidden_states, Wq, Wk, Wv, Wo)
    trace = bool(os.environ.get("BASS_TRACE"))
    res = run_bass_kernel_spmd(nc, in_maps, core_ids=list(range(8)), trace=trace)
    LAST_EXEC_TIME_NS = res.exec_time_ns
    LAST_RESULTS = res
    out = np.concatenate([r["out"] for r in res.results], axis=0)
    return np.ascontiguousarray(out.reshape(2, 4096, H).astype(np.float32))


# revision 31
# speedup vs baseline: 1.0212x; 1.0212x over previous
"""Block2D attention on 8 TRN2 NeuronCores (fp16 compute, fp32 accum).

Sharding: data-parallel over the 8 independent (b, bnx, bny) attention blocks
(B=2 x bnx=2 x bny=2), one block of T=1024 tokens per core. Blocks are fully
independent so no collectives are needed; each core runs the whole
qkv-projection -> block attention -> output projection chain for its block.

Exploits PE array row-tiling concurrency for QK^T: the two heads of a pair
use K=64 contractions on disjoint SBUF partition halves (row groups (0,0)
and (64,0)); emitted back-to-back into disjoint psum banks of one
[128, 2, 512] tile they co-stream on the PE (dispatch ~4ns apart), halving
QK wall time. A single ACT exp drains both heads' scores per (t, n).

The pair loop is software-pipelined: PV runs as two sequential per-head
sweeps (head 1 of pair j-1 during slots 0-3, head 0 of pair j during slots
4-7) so PV holds only 2 psum banks, freeing a dedicated Qproj psum tag —
Qproj(j+1) then executes mid-pair as PE fill while ACT drains exps (a shared
psum ring instead serializes Qproj at pair boundaries and starves ACT
~6.5us/pair). The last pair's head-1 sweep runs inside its own t-loop on the
idle Qproj psum so no serial epilogue delays the output projection. Setup
runs V-proj k-outer with 8 live psum accumulators so matmuls start as soon
as the first xT/Wv k-tile DMA lands.

PSUM budget (8 banks): qk 2 x [128,2,512] (4) + pv 2 x [65,512] (2) +
pp/Qproj 2 x [128,512] (2).
"""

import os
import sys

sys.path.insert(0, "/opt/trn_rl_repo")

import numpy as np
import ml_dtypes

import concourse.bass as bass
from concourse import bacc
import concourse.mybir as mybir
import concourse.tile as tile

F32 = mybir.dt.float32
BF16 = mybir.dt.float16   # compute dtype: fp16 (same PE speed as bf16, 8x finer mantissa)
BF = np.float16

H = 2048        # hidden
T = 1024        # tokens per block
NH = 32         # q heads
NKV = 8         # kv heads
D = 64          # head dim
KT = H // 128   # 16 hidden k-tiles
TT = T // 128   # 8 token tiles
NPAIR = NH // 2  # 16 head pairs
SCALE = D ** -0.5

LAST_EXEC_TIME_NS = None
LAST_RESULTS = None
_CACHED_NC = None


def build_nc(e_bufs=20, wq_bufs=2):
    nc = bacc.Bacc("TRN2")
    xT = nc.dram_tensor("xT", [H, T], BF16, kind="ExternalInput")
    wq = nc.dram_tensor("wq", [H, H], BF16, kind="ExternalInput")
    wk = nc.dram_tensor("wk", [H, NKV * D], BF16, kind="ExternalInput")
    wv = nc.dram_tensor("wv", [H, NKV * D], BF16, kind="ExternalInput")
    wo = nc.dram_tensor("wo", [H, H], BF16, kind="ExternalInput")
    out = nc.dram_tensor("out", [T, H], F32, kind="ExternalOutput")
    scr = nc.dram_tensor("scr", [NPAIR, 2, 2, 512], F32)  # recip bounce [j, hh, n]

    xT_v = xT.ap().rearrange("(k p) t -> p k t", p=128)
    wq_v = wq.ap().rearrange("(k p) m -> p k m", p=128)
    wk_v = wk.ap().rearrange("(k p) m -> p k m", p=128)
    wv_v = wv.ap().rearrange("(k p) m -> p k m", p=128)
    wo_v = wo.ap().rearrange("(k p) m -> p k m", p=128)

    with tile.TileContext(nc) as tc:
        with (
            tc.tile_pool(name="oT", bufs=1) as oT_pool,
            tc.tile_pool(name="xTs", bufs=1) as xT_pool,
            tc.tile_pool(name="kdup", bufs=1) as kdup_pool,
            tc.tile_pool(name="vplus", bufs=1) as vplus_pool,
            tc.tile_pool(name="wos", bufs=2) as wo_pool,
        ):
            oT = oT_pool.tile([128, KT, T], BF16)
            xTs = xT_pool.tile([128, KT, T], BF16)
            kdup = kdup_pool.tile([128, NKV, T], BF16)  # k_g^T on both halves
            vplus = vplus_pool.tile([128, TT, NKV, D + 1], BF16)

            # ---------------- setup: v and k projections ----------------
            # k-outer V projection with 8 live psum accumulators: matmuls
            # start as soon as the first (xT, wv) k-tile DMA lands.
            with (
                tc.tile_pool(name="wvs", bufs=1) as wv_pool,
                tc.tile_pool(name="wks", bufs=2) as wk_pool,
                tc.tile_pool(name="kTs", bufs=1) as kT_pool,
                tc.tile_pool(name="sps", bufs=8, space="PSUM") as sps,
            ):
                wv_s = wv_pool.tile([128, KT, 512], BF16)
                for k in range(KT):
                    nc.sync.dma_start(out=xTs[:, k:k + 1, :], in_=xT_v[:, k:k + 1, :])
                    nc.sync.dma_start(out=wv_s[:, k:k + 1, :], in_=wv_v[:, k:k + 1, :])
                vps = [
                    sps.tile([128, 512], F32, tag="sps", name=f"vps{m}")
                    for m in range(TT)
                ]
                for k in range(KT):
                    for m in range(TT):
                        nc.tensor.matmul(
                            vps[m], xTs[:, k, 128 * m:128 * (m + 1)], wv_s[:, k, :],
                            start=(k == 0), stop=(k == KT - 1),
                        )
                for m in range(TT):
                    nc.vector.tensor_copy(
                        vplus[:, m, :, 0:D],
                        vps[m].rearrange("p (h d) -> p h d", h=NKV),
                    )
                nc.vector.memset(vplus[:, :, :, D:D + 1], 1.0)

                kTs = kT_pool.tile([128, 4, T], BF16)
                for m in range(4):
                    wk_s = wk_pool.tile([128, KT, 128], BF16, tag="wk_s")
                    nc.sync.dma_start(out=wk_s, in_=wk_v[:, :, 128 * m:128 * (m + 1)])
                    for n in range(2):
                        ps = sps.tile([128, 512], F32, tag="sps")
                        for k in range(KT):
                            nc.tensor.matmul(
                                ps, wk_s[:, k, :], xTs[:, k, 512 * n:512 * (n + 1)],
                                start=(k == 0), stop=(k == KT - 1),
                            )
                        nc.vector.tensor_copy(kTs[:, m, 512 * n:512 * (n + 1)], ps)
                    # duplicate the two kv heads of this chunk onto both
                    # partition halves as soon as they exist, so pair 0's QK
                    # can start while later K-proj chunks still run.
                    for g in (2 * m, 2 * m + 1):
                        src = kTs[64 * (g % 2):64 * (g % 2) + 64, g // 2, :]
                        nc.sync.dma_start(out=kdup[0:64, g, :], in_=src)
                        nc.sync.dma_start(out=kdup[64:128, g, :], in_=src)

            # ---------------- pair loop ----------------
            # Per t-slot: QK co-pairs (row-tiled, co-streaming) + exp, plus
            # PV fill work. PV runs as two sequential per-head sweeps so only
            # 2 psum banks are held by PV at any time: head 1 of pair j-1
            # during slots 0-3, head 0 of pair j during slots 4-7. This frees
            # 2 banks for a dedicated Qproj psum tag so Qproj(j+1) runs truly
            # concurrent mid-pair (a shared ring serializes at boundaries and
            # starves ACT ~6.5us/pair).
            with (
                tc.tile_pool(name="wqs", bufs=wq_bufs) as wq_pool,
                tc.tile_pool(name="qpair", bufs=2) as qpair_pool,
                tc.tile_pool(name="e", bufs=e_bufs) as e_pool,
                tc.tile_pool(name="ou", bufs=4) as ou_pool,
                tc.tile_pool(name="rec", bufs=2) as rec_pool,
                tc.tile_pool(name="rbc", bufs=2) as rbc_pool,
                tc.tile_pool(name="qk", bufs=2, space="PSUM") as qk_pool,
                tc.tile_pool(name="pv", bufs=2, space="PSUM") as pv_pool,
                tc.tile_pool(name="pp", bufs=2, space="PSUM") as pp_pool,
            ):
                wo_tiles = [None] * 4
                wo_s0 = wo_pool.tile([128, KT, 512], BF16, tag="wo_s")
                nc.sync.dma_start(out=wo_s0, in_=wo_v[:, :, 0:512])
                wo_tiles[0] = wo_s0

                def emit_qproj(j):
                    """Q projection for pair j -> qpair tile [128, T]."""
                    wq_s = wq_pool.tile([128, KT, 128], BF16, tag="wq_s")
                    nc.sync.dma_start(out=wq_s, in_=wq_v[:, :, 128 * j:128 * (j + 1)])
                    qpair = qpair_pool.tile([128, T], BF16, tag="qpair")
                    for n in range(2):
                        pp = pp_pool.tile([128, 512], F32, tag="pp")
                        for k in range(KT):
                            nc.tensor.matmul(
                                pp, wq_s[:, k, :], xTs[:, k, 512 * n:512 * (n + 1)],
                                start=(k == 0), stop=(k == KT - 1),
                            )
                        nc.vector.tensor_copy(qpair[:, 512 * n:512 * (n + 1)], pp)
                    return qpair

                def emit_evac(j, hh, pv_pair):
                    """Evacuate one head's PV chains: unnormalized o to the
                    pair-packed oT layout, normalized by 1/den (recip +
                    DRAM-bounce broadcast + gpsimd mul, off critical path)."""
                    lo, hi = 64 * hh, 64 * hh + 64
                    # o data sits on partitions 64hh..64hh+64 so the gpsimd
                    # muls see equal base partitions on all three operands.
                    ou_t = ou_pool.tile([128, 2, 512], F32, tag="ou",
                                        name=f"ou_{j}_{hh}")
                    den = rec_pool.tile([1, 2 * 512], F32, tag="den")
                    for n in range(2):
                        nc.vector.tensor_copy(
                            ou_t[lo:hi, n, :], pv_pair[n][0:D, :]
                        )
                        nc.vector.tensor_copy(
                            den[:, 512 * n:512 * (n + 1)], pv_pair[n][D:D + 1, :]
                        )
                    # spread den over 128 partitions so the reciprocal runs
                    # all lanes wide.
                    den_sp = rec_pool.tile([128, 8], F32, tag="den_sp")
                    sp_src = bass.AP(
                        tensor=den.tensor, offset=den.offset,
                        ap=[[1, 1], [8, 128], [1, 8]],
                    )
                    nc.sync.dma_start(out=den_sp, in_=sp_src)
                    rec_sp = rec_pool.tile([128, 8], F32, tag="rec_sp")
                    nc.vector.reciprocal(rec_sp, den_sp)
                    nc.sync.dma_start(
                        out=scr.ap()[j, hh].rearrange("a b -> (a b)")
                        .rearrange("(p e) -> p e", p=128),
                        in_=rec_sp,
                    )
                    rbc = rbc_pool.tile([128, T], F32, tag="rbc")
                    bsrc = bass.AP(
                        tensor=scr.ap().tensor,
                        offset=(2 * j + hh) * T,
                        ap=[[0, 64], [1, 1024]],
                    )
                    nc.sync.dma_start(out=rbc[lo:hi, :], in_=bsrc)
                    for n in range(2):
                        nc.gpsimd.tensor_mul(
                            oT[lo:hi, j, 512 * n:512 * (n + 1)],
                            ou_t[lo:hi, n, :],
                            rbc[lo:hi, 512 * n:512 * (n + 1)],
                        )

                def pv_sweep_steps(j, hh, pv_pair, e_tiles, s0, s1,
                                   pool=None, ptag="pv"):
                    g = j // 2
                    for s in range(s0, s1):
                        for n in range(2):
                            if pv_pair[n] is None:
                                pv_pair[n] = (pool or pv_pool).tile(
                                    [D + 1, 512], F32, tag=ptag,
                                    name=f"pv_{j}_{hh}_{n}",
                                )
                            nc.tensor.matmul(
                                pv_pair[n], vplus[:, s, g, :],
                                e_tiles[s][n][:, hh, :],
                                start=(s == 0), stop=(s == TT - 1),
                            )

                qpair_next = emit_qproj(0)
                e_prev = None      # pair j-1's e tiles, consumed by its h1 sweep
                pv_h1 = [None, None]
                for j in range(NPAIR):
                    g = j // 2
                    qpair = qpair_next
                    e_tiles = [[None] * 2 for _ in range(TT)]  # [t][n]
                    pv_h0 = [None, None]

                    for t in range(TT):
                        for n in range(2):
                            qk = qk_pool.tile([128, 2, 512], F32, tag="qk")
                            for hh in range(2):
                                lo, hi = 64 * hh, 64 * hh + 64
                                nc.tensor.matmul(
                                    qk[:, hh, :],
                                    kdup[lo:hi, g, 128 * t:128 * (t + 1)],
                                    qpair[lo:hi, 512 * n:512 * (n + 1)],
                                    start=True, stop=True,
                                )
                            e = e_pool.tile([128, 2, 512], BF16, tag="e")
                            nc.scalar.activation(
                                e, qk, mybir.ActivationFunctionType.Exp, scale=SCALE
                            )
                            e_tiles[t][n] = e
                        if t < 4:
                            # head-1 sweep of the previous pair (2 steps/slot)
                            if e_prev is not None:
                                pv_sweep_steps(j - 1, 1, pv_h1, e_prev,
                                               2 * t, 2 * t + 2)
                                if t == 3:
                                    emit_evac(j - 1, 1, pv_h1)
                                    pv_h1 = [None, None]
                        else:
                            # head-0 sweep of this pair (2 steps/slot)
                            pv_sweep_steps(j, 0, pv_h0, e_tiles,
                                           2 * (t - 4), 2 * (t - 4) + 2)
                            if t == 4 and j + 1 < NPAIR:
                                qpair_next = emit_qproj(j + 1)
                            if j == NPAIR - 1:
                                # last pair: its head-1 sweep runs inside the
                                # t-loop on the idle Qproj psum (no Qproj(16)),
                                # so no serial epilogue delays the O-proj.
                                pv_sweep_steps(j, 1, pv_h1, e_tiles,
                                               2 * (t - 4), 2 * (t - 4) + 2,
                                               pool=pp_pool, ptag="pp")
                    emit_evac(j, 0, pv_h0)
                    if j == NPAIR - 1:
                        emit_evac(j, 1, pv_h1)
                    e_prev = e_tiles
                    pv_h1 = [None, None]

            # ---------------- output projection ----------------
            with (
                tc.tile_pool(name="ob", bufs=4) as ob_pool,
                tc.tile_pool(name="ops", bufs=4, space="PSUM") as ops,
            ):
                for c in range(4):
                    if wo_tiles[c] is None:
                        wo_s = wo_pool.tile([128, KT, 512], BF16, tag="wo_s")
                        nc.sync.dma_start(
                            out=wo_s, in_=wo_v[:, :, 512 * c:512 * (c + 1)]
                        )
                        wo_tiles[c] = wo_s
                    wo_s = wo_tiles[c]
                    for m in range(TT):
                        ps = ops.tile([128, 512], F32, tag="ops")
                        for k in range(KT):
                            nc.tensor.matmul(
                                ps, oT[:, k, 128 * m:128 * (m + 1)], wo_s[:, k, :],
                                start=(k == 0), stop=(k == KT - 1),
                            )
                        ob = ob_pool.tile([128, 512], F32, tag="ob")
                        nc.scalar.copy(ob, ps)
                        nc.sync.dma_start(
                            out=out.ap()[128 * m:128 * (m + 1), 512 * c:512 * (c + 1)],
                            in_=ob,
                        )
    nc.finalize()
    return nc


def _prep_inputs(hidden_states, Wq, Wk, Wv, Wo):
    hs = np.asarray(hidden_states, dtype=np.float32)
    B = hs.shape[0]
    # token index l = ix*2048 + sx*64 + iy*32 + sy  (bnx=2, BSX=32, bny=2, BSY=32)
    hsv = hs.reshape(B, 2, 32, 2, 32, H)  # b ix sx iy sy h
    wq_b = np.asarray(Wq, dtype=np.float32).astype(BF)
    wk_b = np.asarray(Wk, dtype=np.float32).astype(BF)
    wv_b = np.asarray(Wv, dtype=np.float32).astype(BF)
    wo_b = np.asarray(Wo, dtype=np.float32).astype(BF)
    in_maps = []
    for c in range(8):
        b, ix, iy = c // 4, (c // 2) % 2, c % 2
        x_blk = hsv[b, ix, :, iy, :, :].reshape(T, H)
        xT = np.ascontiguousarray(x_blk.T).astype(BF)
        in_maps.append({"xT": xT, "wq": wq_b, "wk": wk_b, "wv": wv_b, "wo": wo_b})
    return in_maps


def kernel(hidden_states, Wq, Wk, Wv, Wo, x_dim=64, y_dim=64):
    global LAST_EXEC_TIME_NS, LAST_RESULTS, _CACHED_NC
    assert int(x_dim) == 64 and int(y_dim) == 64

    from concourse.bass_utils import run_bass_kernel_spmd

    if _CACHED_NC is None:
        _CACHED_NC = build_nc()
    nc = _CACHED_NC

    in_maps = _prep_inputs(hidden_states, Wq, Wk, Wv, Wo)
    trace = bool(os.environ.get("BASS_TRACE"))
    res = run_bass_kernel_spmd(nc, in_maps, core_ids=list(range(8)), trace=trace)
    LAST_EXEC_TIME_NS = res.exec_time_ns
    LAST_RESULTS = res
    out = np.concatenate([r["out"] for r in res.results], axis=0)
    return np.ascontiguousarray(out.reshape(2, 4096, H).astype(np.float32))
